# revision 23
# baseline (speedup 1.0000x reference)
"""Trainium2 Bass kernel for AdaptivePseudoLabelRefinement (retrieval-KNN).

Pipeline (8 NeuronCores, full inputs in / full outputs out):
  L1 (device, feature-dim sharded): stream the 512x131072 feature queue
      (fp8e4, selection-only precision) through the PE array to get, per
      core, partial [q_norm, dot(q, t0), dot(q, t1)] over its
      16384-feature shard. Dots use DoubleRow fp8 matmuls (col-group 0),
      norms a concurrent ones-matmul in col-group 1.
  host glue (tiny): sum partials -> approx distances -> top-128
      candidate rows -> exact fp64 re-verification -> argmin row and
      closest distance; exact d2 of the 32 augmented features vs the
      closest row -> mask / top-k / weights exactly mirroring the
      reference semantics.
  L2 (device, spatial-row sharded): for the k selected augmentations,
      softmax over classes (pixel-major layout) and 0/1-weighted
      PSUM accumulation -> unnormalized average U; also the target-logits
      confidence mask. Host divides by max(count,1), argmaxes, and
      assembles the refined labels exactly like the reference.
"""

import os
import time
import numpy as np

import ml_dtypes

Q = 512          # queue rows
D = 512 * 16 * 16  # 131072 feature dim
B = 2            # batch
A = 16           # augmentations
C = 19           # classes
H = 256
W = 256
N_CORES = 8
DSH = D // N_CORES      # 16384 features per core (L1 shard)
HSH = H // N_CORES      # 32 rows per core (L2 shard)
PXS = HSH * W           # 8192 pixels per core per (b, slab)
L = PXS // 128          # 64 pixel groups per partition
REFINE_CONF = 0.968
N_CAND = 128            # candidate rows re-verified exactly on host

_MODULES = {}
LAST_TIMING = {}


def _bass_imports():
    import concourse.bacc as bacc
    import concourse.mybir as mybir
    import concourse.tile as tile
    from concourse import bass_utils
    from concourse.bass_interp import get_hw_module
    return bacc, mybir, tile, bass_utils, get_hw_module


def _build_l1():
    """Queue-distance partials. Per core: qT [NJ, 128, SC, Q] fp8e4
    (transposed queue shard in on-chip tile order), tT [128, NPAIR, 2,
    PADM] fp8e4 (DoubleRow-interleaved target shard). Output part [3, Q]
    f32 = [q_norm, dot_t0, dot_t1].

    The dot (DoubleRow) and norm matmul streams run concurrently in
    distinct PE column-groups (tile_position col 0 vs 32), and the queue
    streams in fp8 (selection-only precision: the host re-verifies the
    top candidates exactly)."""
    bacc, mybir, tile, bass_utils, get_hw_module = _bass_imports()
    fp8 = mybir.dt.float8e4
    f32 = mybir.dt.float32

    NJ = 16                    # DMA loads
    SC = DSH // 128 // NJ      # 8 d-chunks of 128 per load
    NCHUNK = DSH // 128        # 128
    NPAIR = NCHUNK // 2        # DoubleRow processes chunk pairs
    N_ACT = 9                  # squares: 9/16 of sub-blocks on ACT, rest DVE
    SQ_SPLIT = 4               # square sub-blocks per load (pipelining grain)

    PADM = 16                  # DoubleRow weight Ko-step must be 16 bytes

    nc = bacc.Bacc("TRN2", target_bir_lowering=False, debug=False,
                   num_devices=N_CORES)
    # host pre-arranges the queue in on-chip tile order so every partition
    # reads one fully contiguous 4KB run per load
    qT = nc.dram_tensor("qT", [NJ, 128, SC, Q], fp8, kind="ExternalInput")
    # target shard interleaved for DoubleRow: (p, pair, ko, m) with the two
    # targets in m = 0, 1 and zero padding up to PADM
    tT = nc.dram_tensor("tT", [128, NPAIR, 2, PADM], fp8,
                        kind="ExternalInput")
    out = nc.dram_tensor("part", [3, Q], f32, kind="ExternalOutput")
    PM = mybir.MatmulPerfMode.DoubleRow

    with tile.TileContext(nc) as tc:
        with (
            tc.tile_pool(name="const", bufs=1) as constp,
            tc.tile_pool(name="qload", bufs=5) as qp,
            tc.tile_pool(name="sqp", bufs=5) as sqp,
            tc.tile_pool(name="psum", bufs=1, space="PSUM") as pp,
            tc.tile_pool(name="outp", bufs=1) as op,
        ):
            tTs = constp.tile([128, NPAIR, 2, PADM], fp8)
            nc.sync.dma_start(tTs[:], tT.ap())
            ones = constp.tile([128, 1], fp8)
            nc.vector.memset(ones[:], 1.0)

            # dots: DoubleRow (2 chunks per matmul) into partitions 0-15 of
            # PE col-group 0; norms: plain fp8 ones-matmul into partition 32
            # (col-group 1) so the two streams overlap in separate column
            # strips of the array. Separate PSUM banks (one accumulation
            # group per bank).
            psd = pp.tile([PADM, Q], f32, tag="psd")
            psn = pp.tile([33, Q], f32, tag="psn")

            qv = qT.ap()
            for j in range(NJ):
                tq = qp.tile([128, SC, Q], fp8)
                nc.sync.dma_start(tq[:], qv[j])
                sq = sqp.tile([128, SC, Q], fp8)
                for z in range(SQ_SPLIT):
                    z0 = z * SC // SQ_SPLIT
                    z1 = (z + 1) * SC // SQ_SPLIT
                    if (j * SQ_SPLIT + z) % 16 < N_ACT:
                        nc.scalar.square(sq[:, z0:z1], tq[:, z0:z1])
                    else:
                        nc.vector.tensor_mul(sq[:, z0:z1], tq[:, z0:z1],
                                             tq[:, z0:z1])
                for s in range(0, SC, 2):
                    pi = (j * SC + s) // 2
                    nc.tensor.matmul(psd[:, :], tTs[:, pi, :, :],
                                     tq[:, s:s + 2, :],
                                     start=(pi == 0), stop=(pi == NPAIR - 1),
                                     perf_mode=PM, tile_position=(0, 0))
                for s in range(SC):
                    ci = j * SC + s
                    nc.tensor.matmul(psn[32:33, :], ones[:], sq[:, s, :],
                                     start=(ci == 0), stop=(ci == NCHUNK - 1),
                                     tile_position=(0, 32))

            outs_d = op.tile([2, Q], f32, tag="outd")
            nc.vector.tensor_copy(outs_d[:], psd[0:B, :])
            nc.sync.dma_start(out.ap()[1:3], outs_d[:])
            outs_n = op.tile([33, Q], f32, tag="outn")
            nc.vector.tensor_copy(outs_n[32:33, :], psn[32:33, :])
            nc.sync.dma_start(out.ap()[0:1], outs_n[32:33, :])

    nc.compile()
    return nc


def _build_l2(k):
    """Weighted softmax average over the k selected augmentations plus the
    target-logits confidence mask, pixel-major layout, H-sharded.

    Per core: g [B*k, PXS, C] f32 (gathered aug logits), wI [B*k, 128, 128]
    f32 (weight * identity, stationary for exact fp32 PSUM accumulation),
    tg [B, PXS, C] f32 (target logits).
    Outputs: u [B, PXS, C] f32 (sum_j w_j * softmax_j), conf [B, PXS] f32
    (1.0 where max softmax < REFINE_CONF)."""
    bacc, mybir, tile, bass_utils, get_hw_module = _bass_imports()
    f32 = mybir.dt.float32
    AF = mybir.ActivationFunctionType
    ALU = mybir.AluOpType
    AX = mybir.AxisListType
    S = B * k

    nc = bacc.Bacc("TRN2", target_bir_lowering=False, debug=False,
                   num_devices=N_CORES)
    g = nc.dram_tensor("g", [S, PXS, C], f32, kind="ExternalInput")
    wI = nc.dram_tensor("wI", [S, 128, 128], f32, kind="ExternalInput")
    tg = nc.dram_tensor("tg", [B, PXS, C], f32, kind="ExternalInput")
    u = nc.dram_tensor("u", [B, PXS, C], f32, kind="ExternalOutput")
    cf = nc.dram_tensor("conf", [B, PXS], f32, kind="ExternalOutput")

    gview = g.ap().rearrange("s (p l) c -> s p l c", p=128)
    tview = tg.ap().rearrange("b (p l) c -> b p l c", p=128)
    uview = u.ap().rearrange("b (p l) c -> b p l c", p=128)
    cview = cf.ap().rearrange("b (p l) -> b p l", p=128)
    FD = L * C  # 1216 accumulator columns per partition
    NSPLIT = [(0, 512), (512, 1024), (1024, FD)]

    with tile.TileContext(nc) as tc:
        with (
            tc.tile_pool(name="const", bufs=1) as constp,
            tc.tile_pool(name="raw", bufs=3) as rawp,
            tc.tile_pool(name="exp", bufs=3) as ep,
            tc.tile_pool(name="small", bufs=4) as sp,
            tc.tile_pool(name="smp", bufs=3) as smp,
            tc.tile_pool(name="psum", bufs=1, space="PSUM") as pp,
            tc.tile_pool(name="accp", bufs=2) as accp,
        ):
            # per-slab 0/1-weighted identity: stationary for PE accumulation
            wIs = constp.tile([128, S, 128], f32)
            nc.sync.dma_start(wIs[:], wI.ap().rearrange("s p m -> p s m"))

            for b in range(B):
                acc_ps = pp.tile([128, FD], f32, tag=f"acc_{b}")
                for j in range(k):
                    s = b * k + j
                    raw = rawp.tile([128, L, C], f32)
                    nc.sync.dma_start(raw[:], gview[s])
                    E = ep.tile([128, L, C], f32)
                    nc.scalar.activation(E[:], raw[:], AF.Exp)
                    Ssum = sp.tile([128, L], f32)
                    nc.vector.tensor_reduce(Ssum[:], E[:], AX.X, ALU.add)
                    R = sp.tile([128, L], f32, tag="recip")
                    nc.vector.reciprocal(R[:], Ssum[:])
                    sm = smp.tile([128, L, C], f32)
                    rb = R[:].unsqueeze(-1).broadcast_to([128, L, C])
                    nc.vector.tensor_tensor(sm[:], E[:], rb, ALU.mult)
                    sm_flat = sm[:].rearrange("p l c -> p (l c)")
                    for n0, n1 in NSPLIT:
                        nc.tensor.matmul(acc_ps[:, n0:n1], wIs[:, s, :],
                                         sm_flat[:, n0:n1],
                                         start=(j == 0), stop=(j == k - 1))
                Uacc = accp.tile([128, FD], f32)
                nc.vector.tensor_copy(Uacc[:], acc_ps[:])
                nc.sync.dma_start(
                    uview[b], Uacc[:].rearrange("p (l c) -> p l c", c=C))

                # target-logits confidence: 1.0 where max softmax < conf
                rawt = rawp.tile([128, L, C], f32)
                nc.sync.dma_start(rawt[:], tview[b])
                Et = ep.tile([128, L, C], f32)
                nc.scalar.activation(Et[:], rawt[:], AF.Exp)
                mx = sp.tile([128, L], f32, tag="mx")
                nc.vector.tensor_reduce(mx[:], Et[:], AX.X, ALU.max)
                sm_ = sp.tile([128, L], f32, tag="sum")
                nc.vector.tensor_reduce(sm_[:], Et[:], AX.X, ALU.add)
                cmpo = sp.tile([128, L], f32, tag="cmp")
                nc.vector.scalar_tensor_tensor(
                    cmpo[:], sm_[:], REFINE_CONF, mx[:], ALU.mult, ALU.is_gt)
                nc.sync.dma_start(cview[b], cmpo[:])

    nc.compile()
    return nc


def _get_module(name, k=None):
    key = (name, k)
    if key not in _MODULES:
        if name == "l1":
            _MODULES[key] = _build_l1()
        else:
            _MODULES[key] = _build_l2(k)
    return _MODULES[key]


def _run_hw(nc, in_maps):
    """Run the compiled module on the 8 NeuronCores via PJRT/axon."""
    _, _, _, bass_utils, get_hw_module = _bass_imports()
    from concourse.bass_interp import get_hw_module as _ghm
    old_m = nc.m
    nc.m = _ghm(nc.m)
    try:
        t0 = time.perf_counter()
        res = bass_utils.run_bass_kernel_spmd(
            nc, in_maps, core_ids=list(range(N_CORES)))
        t1 = time.perf_counter()
    finally:
        nc.m = old_m
    return res.results, t1 - t0


def _run_sim(nc, in_maps):
    """CoreSim fallback (KERNEL_BACKEND=sim) for development."""
    from concourse.bass_interp import CoreSim
    results = []
    for m in in_maps:
        sim = CoreSim(nc, require_finite=False)
        for name, arr in m.items():
            sim.tensor(name)[:] = arr
        sim.simulate(check_with_hw=False)
        outs = {}
        for alloc in nc.m.functions[0].allocations:
            try:
                kind = alloc.kind
            except AttributeError:
                continue
            if kind == "ExternalOutput":
                nm = alloc.memorylocations[0].name
                outs[nm] = np.array(sim.tensor(nm))
        results.append(outs)
    return results, 0.0


def _run(nc, in_maps):
    if os.environ.get("KERNEL_BACKEND") == "sim":
        return _run_sim(nc, in_maps)
    return _run_hw(nc, in_maps)


def kernel(source_queue, tgt_feat, tgt_logits, auged_feat, auged_logits,
           pseudo_label, k):
    k = int(k)
    fp8 = ml_dtypes.float8_e4m3

    source_queue = np.ascontiguousarray(np.asarray(source_queue, dtype=np.float32))
    tgt_feat = np.asarray(tgt_feat, dtype=np.float32)
    tgt_logits = np.asarray(tgt_logits, dtype=np.float32)
    auged_feat = np.asarray(auged_feat, dtype=np.float32)
    auged_logits = np.ascontiguousarray(np.asarray(auged_logits, dtype=np.float32))
    pseudo_label = np.asarray(pseudo_label)

    qflat = source_queue.reshape(Q, D)
    tflat = tgt_feat.reshape(B, D)

    # ---------------- L1: queue distance partials (device) ----------------
    t_prep0 = time.perf_counter()
    nc1 = _get_module("l1")
    in_maps1 = []
    NJ = 16
    SC = DSH // 128 // NJ
    for c in range(N_CORES):
        c0 = c * DSH
        # [DSH, Q] transposed shard, rows d = j*(SC*128) + s*128 + p,
        # rearranged to [NJ, 128, SC, Q] so partition reads are contiguous
        qT_c = np.ascontiguousarray(
            qflat[:, c0:c0 + DSH].T.reshape(NJ, SC, 128, Q)
            .transpose(0, 2, 1, 3)).astype(fp8)
        # DoubleRow weight layout: (p, pair, ko, m), d = (pair*2+ko)*128 + p,
        # targets at m = 0..B-1, zero-padded to 16 columns
        tT_c = np.zeros((128, DSH // 256, 2, 16), dtype=np.float32)
        tT_c[:, :, :, :B] = (tflat[:, c0:c0 + DSH]
                             .reshape(B, DSH // 256, 2, 128)
                             .transpose(3, 1, 2, 0))
        in_maps1.append({"qT": qT_c, "tT": tT_c.astype(fp8)})
    t_prep1 = time.perf_counter()

    res1, l1_wall = _run(nc1, in_maps1)

    parts = np.stack([r["part"] for r in res1]).astype(np.float64)  # [8,3,Q]
    psum = parts.sum(axis=0)
    qn_dev = psum[0]
    dots_dev = psum[1:3]  # [B, Q] -- dots_dev[b, r] = q_r . t_b

    # ------------- host glue: exact selection (tiny, fp64) ---------------
    t64 = tflat.astype(np.float64)
    tn = (t64 ** 2).sum(axis=1)
    i_min = np.zeros(B, dtype=np.int64)
    cd = np.zeros(B)
    for b in range(B):
        d2_approx = qn_dev - 2.0 * dots_dev[b] + tn[b]
        cands = np.argsort(d2_approx, kind="stable")[:N_CAND]
        qc = qflat[cands].astype(np.float64)
        exact = ((qc - t64[b]) ** 2).sum(axis=1)
        j = int(np.argmin(exact))
        i_min[b] = cands[j]
        cd[b] = np.sqrt(exact[j])

    af64 = auged_feat.reshape(B, A, D).astype(np.float64)
    idx = np.zeros((B, k), dtype=np.int64)
    wsel = np.zeros((B, k), dtype=np.float32)
    counts = np.zeros(B, dtype=np.int64)
    denom = np.ones(B, dtype=np.float32)
    for b in range(B):
        closest = qflat[i_min[b]].astype(np.float64)
        d2 = np.sqrt(((af64[b] - closest[None, :]) ** 2).sum(axis=1))  # [A]
        mask = d2 <= cd[b]
        counts[b] = int(mask.sum())
        masked = np.where(mask, d2, np.inf)
        order = np.argsort(masked, kind="stable")[:k]
        idx[b] = order
        wsel[b] = np.isfinite(masked[order]).astype(np.float32)
        denom[b] = max(float(wsel[b].sum()), 1.0)

    # ---------------- L2: weighted softmax average (device) ----------------
    t_prep2 = time.perf_counter()
    nc2 = _get_module("l2", k)
    sel = auged_logits[np.arange(B)[:, None], idx]  # [B, k, C, H, W]
    eye = np.eye(128, dtype=np.float32)
    wI_np = (wsel.reshape(B * k, 1, 1) * eye[None]).astype(np.float32)
    in_maps2 = []
    for c in range(N_CORES):
        h0 = c * HSH
        g_c = np.ascontiguousarray(
            sel[:, :, :, h0:h0 + HSH, :].transpose(0, 1, 3, 4, 2)
        ).reshape(B * k, PXS, C)
        tg_c = np.ascontiguousarray(
            tgt_logits[:, :, h0:h0 + HSH, :].transpose(0, 2, 3, 1)
        ).reshape(B, PXS, C)
        in_maps2.append({"g": g_c, "wI": wI_np, "tg": tg_c})
    t_prep3 = time.perf_counter()

    res2, l2_wall = _run(nc2, in_maps2)

    # ---------------- host: assemble + final label logic ----------------
    U = np.empty((B, H, W, C), dtype=np.float32)
    conf = np.empty((B, H, W), dtype=np.float32)
    for c in range(N_CORES):
        h0 = c * HSH
        U[:, h0:h0 + HSH] = res2[c]["u"].reshape(B, HSH, W, C)
        conf[:, h0:h0 + HSH] = res2[c]["conf"].reshape(B, HSH, W)

    avg_soft = np.ascontiguousarray(
        (U / denom[:, None, None, None]).transpose(0, 3, 1, 2))  # [B, C, H, W]

    knn = np.argmax(avg_soft, axis=1).astype(pseudo_label.dtype)
    labels = np.where((counts > 0)[:, None, None], knn, pseudo_label)
    pseudo_mask = conf > 0.5
    refined = np.where(pseudo_mask, labels, pseudo_label).astype(pseudo_label.dtype)

    LAST_TIMING.update(dict(
        l1_wall=l1_wall, l2_wall=l2_wall,
        prep1=t_prep1 - t_prep0, prep2=t_prep3 - t_prep2))

    return refined, avg_soft


# revision 29
# speedup vs baseline: 1.0877x; 1.0877x over previous
"""Trainium2 Bass kernel for AdaptivePseudoLabelRefinement (retrieval-KNN).

Pipeline (8 NeuronCores, full inputs in / full outputs out):
  L1 (device, feature-dim sharded): stream the 512x131072 feature queue
      (fp8e4, selection-only precision) through the PE array to get, per
      core, partial [q_norm, dot(q, t0), dot(q, t1)] over its
      16384-feature shard. Dots use DoubleRow fp8 matmuls (col-group 0),
      norms a concurrent ones-matmul in col-group 1.
  host glue (tiny): sum partials -> approx distances -> top-128
      candidate rows -> exact fp64 re-verification -> argmin row and
      closest distance; exact d2 of the 32 augmented features vs the
      closest row -> mask / top-k / weights exactly mirroring the
      reference semantics.
  L2 (device, spatial-row sharded): for the k selected augmentations,
      softmax over classes (pixel-major layout) and 0/1-weighted
      PSUM accumulation -> unnormalized average U; also the target-logits
      confidence mask. Host divides by max(count,1), argmaxes, and
      assembles the refined labels exactly like the reference.
"""

import os
import time
import numpy as np

import ml_dtypes

Q = 512          # queue rows
D = 512 * 16 * 16  # 131072 feature dim
B = 2            # batch
A = 16           # augmentations
C = 19           # classes
H = 256
W = 256
N_CORES = 8
DSH = D // N_CORES      # 16384 features per core (L1 shard)
HSH = H // N_CORES      # 32 rows per core (L2 shard)
PXS = HSH * W           # 8192 pixels per core per (b, slab)
L = PXS // 128          # 64 pixel groups per partition
REFINE_CONF = 0.968
N_CAND = 128            # candidate rows re-verified exactly on host

_MODULES = {}
LAST_TIMING = {}


def _bass_imports():
    import concourse.bacc as bacc
    import concourse.mybir as mybir
    import concourse.tile as tile
    from concourse import bass_utils
    from concourse.bass_interp import get_hw_module
    return bacc, mybir, tile, bass_utils, get_hw_module


def _build_l1():
    """Queue-distance partials. Per core: qT [NJ, 128, SC, Q] fp8e4
    (transposed queue shard in on-chip tile order), tT [128, NPAIR, 2,
    PADM] fp8e4 (DoubleRow-interleaved target shard). Output part [3, Q]
    f32 = [q_norm, dot_t0, dot_t1].

    The dot (DoubleRow) and norm matmul streams run concurrently in
    distinct PE column-groups (tile_position col 0 vs 32), and the queue
    streams in fp8 (selection-only precision: the host re-verifies the
    top candidates exactly)."""
    bacc, mybir, tile, bass_utils, get_hw_module = _bass_imports()
    fp8 = mybir.dt.float8e4
    f32 = mybir.dt.float32

    NJ = 16                    # DMA loads
    SC = DSH // 128 // NJ      # 8 d-chunks of 128 per load
    NCHUNK = DSH // 128        # 128
    NPAIR = NCHUNK // 2        # DoubleRow processes chunk pairs
    N_ACT = 8                  # squares: 8/16 of sub-blocks on ACT, rest DVE
    SQ_SPLIT = 4               # square sub-blocks per load (pipelining grain)

    PADM = 16                  # DoubleRow weight Ko-step must be 16 bytes

    nc = bacc.Bacc("TRN2", target_bir_lowering=False, debug=False,
                   num_devices=N_CORES)
    # host pre-arranges the queue in on-chip tile order so every partition
    # reads one fully contiguous 4KB run per load
    qT = nc.dram_tensor("qT", [NJ, 128, SC, Q], fp8, kind="ExternalInput")
    # target shard interleaved for DoubleRow: (p, pair, ko, m) with the two
    # targets in m = 0, 1 and zero padding up to PADM
    tT = nc.dram_tensor("tT", [128, NPAIR, 2, PADM], fp8,
                        kind="ExternalInput")
    out = nc.dram_tensor("part", [3, Q], f32, kind="ExternalOutput")
    PM = mybir.MatmulPerfMode.DoubleRow

    with tile.TileContext(nc) as tc:
        with (
            tc.tile_pool(name="const", bufs=1) as constp,
            tc.tile_pool(name="qload", bufs=5) as qp,
            tc.tile_pool(name="sqp", bufs=5) as sqp,
            tc.tile_pool(name="psum", bufs=1, space="PSUM") as pp,
            tc.tile_pool(name="outp", bufs=1) as op,
        ):
            tTs = constp.tile([128, NPAIR, 2, PADM], fp8)
            nc.sync.dma_start(tTs[:], tT.ap())
            ones = constp.tile([128, 1], fp8)
            nc.vector.memset(ones[:], 1.0)

            # dots: DoubleRow (2 chunks per matmul) into partitions 0-15 of
            # PE col-group 0; norms: plain fp8 ones-matmul into partition 32
            # (col-group 1) so the two streams overlap in separate column
            # strips of the array. Separate PSUM banks (one accumulation
            # group per bank).
            psd = pp.tile([PADM, Q], f32, tag="psd")
            psn = pp.tile([33, Q], f32, tag="psn")

            qv = qT.ap()
            for j in range(NJ):
                tq = qp.tile([128, SC, Q], fp8)
                nc.sync.dma_start(tq[:], qv[j])
                sq = sqp.tile([128, SC, Q], fp8)
                for z in range(SQ_SPLIT):
                    z0 = z * SC // SQ_SPLIT
                    z1 = (z + 1) * SC // SQ_SPLIT
                    if (j * SQ_SPLIT + z) % 16 < N_ACT:
                        nc.scalar.square(sq[:, z0:z1], tq[:, z0:z1])
                    else:
                        nc.vector.tensor_mul(sq[:, z0:z1], tq[:, z0:z1],
                                             tq[:, z0:z1])
                for s in range(0, SC, 2):
                    pi = (j * SC + s) // 2
                    nc.tensor.matmul(psd[:, :], tTs[:, pi, :, :],
                                     tq[:, s:s + 2, :],
                                     start=(pi == 0), stop=(pi == NPAIR - 1),
                                     perf_mode=PM, tile_position=(0, 0))
                for s in range(SC):
                    ci = j * SC + s
                    nc.tensor.matmul(psn[32:33, :], ones[:], sq[:, s, :],
                                     start=(ci == 0), stop=(ci == NCHUNK - 1),
                                     tile_position=(0, 32))

            outs_d = op.tile([2, Q], f32, tag="outd")
            nc.vector.tensor_copy(outs_d[:], psd[0:B, :])
            nc.sync.dma_start(out.ap()[1:3], outs_d[:])
            outs_n = op.tile([33, Q], f32, tag="outn")
            nc.vector.tensor_copy(outs_n[32:33, :], psn[32:33, :])
            nc.sync.dma_start(out.ap()[0:1], outs_n[32:33, :])

    nc.compile()
    return nc


def _build_l2(k):
    """Weighted softmax average over the k selected augmentations plus the
    target-logits confidence mask, pixel-major layout, H-sharded.

    Per core: g [B*k, PXS, C] f32 (gathered aug logits), wI [B*k, 128, 128]
    f32 (weight * identity, stationary for exact fp32 PSUM accumulation),
    tg [B, PXS, C] f32 (target logits).
    Outputs: u [B, PXS, C] f32 (sum_j w_j * softmax_j), conf [B, PXS] f32
    (1.0 where max softmax < REFINE_CONF)."""
    bacc, mybir, tile, bass_utils, get_hw_module = _bass_imports()
    f32 = mybir.dt.float32
    AF = mybir.ActivationFunctionType
    ALU = mybir.AluOpType
    AX = mybir.AxisListType
    S = B * k

    nc = bacc.Bacc("TRN2", target_bir_lowering=False, debug=False,
                   num_devices=N_CORES)
    g = nc.dram_tensor("g", [S, PXS, C], f32, kind="ExternalInput")
    wI = nc.dram_tensor("wI", [S, 128, 128], f32, kind="ExternalInput")
    tg = nc.dram_tensor("tg", [B, PXS, C], f32, kind="ExternalInput")
    u = nc.dram_tensor("u", [B, PXS, C], f32, kind="ExternalOutput")
    cf = nc.dram_tensor("conf", [B, PXS], f32, kind="ExternalOutput")

    gview = g.ap().rearrange("s (p l) c -> s p l c", p=128)
    tview = tg.ap().rearrange("b (p l) c -> b p l c", p=128)
    uview = u.ap().rearrange("b (p l) c -> b p l c", p=128)
    cview = cf.ap().rearrange("b (p l) -> b p l", p=128)
    FD = L * C  # 1216 accumulator columns per partition
    # sub-slab pipelining: normalize and accumulate in l-aligned thirds so
    # the PE accumulation of one third overlaps the DVE normalize of the
    # next. Each third's PSUM region starts on its own bank (512-f32
    # aligned) because a matmul output cannot cross a bank boundary.
    LSPLIT = [(0, 26), (26, 52), (52, L)]
    BANK = 512
    ACC_OFF = [0 * BANK, 1 * BANK, 2 * BANK]

    with tile.TileContext(nc) as tc:
        with (
            tc.tile_pool(name="const", bufs=1) as constp,
            tc.tile_pool(name="raw", bufs=3) as rawp,
            tc.tile_pool(name="exp", bufs=3) as ep,
            tc.tile_pool(name="small", bufs=4) as sp,
            tc.tile_pool(name="smp", bufs=4) as smp,
            tc.tile_pool(name="psum", bufs=1, space="PSUM") as pp,
            tc.tile_pool(name="accp", bufs=2) as accp,
        ):
            # per-slab 0/1-weighted identity: stationary for PE accumulation
            wIs = constp.tile([128, S, 128], f32)
            nc.sync.dma_start(wIs[:], wI.ap().rearrange("s p m -> p s m"))

            accs = {}
            for b in range(B):
                acc_t = pp.tile([128, 2 * BANK + (FD - 52 * C)], f32,
                                tag=f"acc_{b}")
                accs[b] = acc_t

            # interleave the two batch samples' slab streams so their PSUM
            # accumulations and DVE chains overlap
            for j in range(k):
                for b in range(B):
                    s = b * k + j
                    raw = rawp.tile([128, L, C], f32)
                    nc.sync.dma_start(raw[:], gview[s])
                    E = ep.tile([128, L, C], f32)
                    nc.scalar.activation(E[:], raw[:], AF.Exp)
                    Ssum = sp.tile([128, L], f32)
                    R = sp.tile([128, L], f32, tag="recip")
                    sm = smp.tile([128, L, C], f32)
                    sm_flat = sm[:].rearrange("p l c -> p (l c)")
                    for t, (l0, l1) in enumerate(LSPLIT):
                        nc.vector.tensor_reduce(Ssum[:, l0:l1],
                                                E[:, l0:l1, :], AX.X, ALU.add)
                        nc.vector.reciprocal(R[:, l0:l1], Ssum[:, l0:l1])
                        rb = R[:, l0:l1].unsqueeze(-1).broadcast_to(
                            [128, l1 - l0, C])
                        nc.vector.tensor_tensor(sm[:, l0:l1, :],
                                                E[:, l0:l1, :], rb, ALU.mult)
                        o = ACC_OFF[t]
                        nc.tensor.matmul(
                            accs[b][:, o:o + (l1 - l0) * C],
                            wIs[:, s, :], sm_flat[:, l0 * C:l1 * C],
                            start=(j == 0), stop=(j == k - 1))

            for b in range(B):
                Uacc = accp.tile([128, FD], f32)
                for t, (l0, l1) in enumerate(LSPLIT):
                    o = ACC_OFF[t]
                    nc.vector.tensor_copy(
                        Uacc[:, l0 * C:l1 * C],
                        accs[b][:, o:o + (l1 - l0) * C])
                nc.sync.dma_start(
                    uview[b], Uacc[:].rearrange("p (l c) -> p l c", c=C))

            # target-logits confidence: 1.0 where max softmax < conf
            for b in range(B):
                rawt = rawp.tile([128, L, C], f32, tag="rawt")
                nc.sync.dma_start(rawt[:], tview[b])
                Et = ep.tile([128, L, C], f32, tag="Et")
                nc.scalar.activation(Et[:], rawt[:], AF.Exp)
                mx = sp.tile([128, L], f32, tag="mx")
                nc.vector.tensor_reduce(mx[:], Et[:], AX.X, ALU.max)
                sm_ = sp.tile([128, L], f32, tag="sum")
                nc.vector.tensor_reduce(sm_[:], Et[:], AX.X, ALU.add)
                cmpo = sp.tile([128, L], f32, tag="cmp")
                nc.vector.scalar_tensor_tensor(
                    cmpo[:], sm_[:], REFINE_CONF, mx[:], ALU.mult, ALU.is_gt)
                nc.sync.dma_start(cview[b], cmpo[:])

    nc.compile()
    return nc


def _get_module(name, k=None):
    key = (name, k)
    if key not in _MODULES:
        if name == "l1":
            _MODULES[key] = _build_l1()
        else:
            _MODULES[key] = _build_l2(k)
    return _MODULES[key]


def _run_hw(nc, in_maps):
    """Run the compiled module on the 8 NeuronCores via PJRT/axon."""
    _, _, _, bass_utils, get_hw_module = _bass_imports()
    from concourse.bass_interp import get_hw_module as _ghm
    old_m = nc.m
    nc.m = _ghm(nc.m)
    try:
        t0 = time.perf_counter()
        res = bass_utils.run_bass_kernel_spmd(
            nc, in_maps, core_ids=list(range(N_CORES)))
        t1 = time.perf_counter()
    finally:
        nc.m = old_m
    return res.results, t1 - t0


def _run_sim(nc, in_maps):
    """CoreSim fallback (KERNEL_BACKEND=sim) for development."""
    from concourse.bass_interp import CoreSim
    results = []
    for m in in_maps:
        sim = CoreSim(nc, require_finite=False)
        for name, arr in m.items():
            sim.tensor(name)[:] = arr
        sim.simulate(check_with_hw=False)
        outs = {}
        for alloc in nc.m.functions[0].allocations:
            try:
                kind = alloc.kind
            except AttributeError:
                continue
            if kind == "ExternalOutput":
                nm = alloc.memorylocations[0].name
                outs[nm] = np.array(sim.tensor(nm))
        results.append(outs)
    return results, 0.0


def _run(nc, in_maps):
    if os.environ.get("KERNEL_BACKEND") == "sim":
        return _run_sim(nc, in_maps)
    return _run_hw(nc, in_maps)


def kernel(source_queue, tgt_feat, tgt_logits, auged_feat, auged_logits,
           pseudo_label, k):
    k = int(k)
    fp8 = ml_dtypes.float8_e4m3

    source_queue = np.ascontiguousarray(np.asarray(source_queue, dtype=np.float32))
    tgt_feat = np.asarray(tgt_feat, dtype=np.float32)
    tgt_logits = np.asarray(tgt_logits, dtype=np.float32)
    auged_feat = np.asarray(auged_feat, dtype=np.float32)
    auged_logits = np.ascontiguousarray(np.asarray(auged_logits, dtype=np.float32))
    pseudo_label = np.asarray(pseudo_label)

    qflat = source_queue.reshape(Q, D)
    tflat = tgt_feat.reshape(B, D)

    # ---------------- L1: queue distance partials (device) ----------------
    t_prep0 = time.perf_counter()
    nc1 = _get_module("l1")
    in_maps1 = []
    NJ = 16
    SC = DSH // 128 // NJ
    for c in range(N_CORES):
        c0 = c * DSH
        # [DSH, Q] transposed shard, rows d = j*(SC*128) + s*128 + p,
        # rearranged to [NJ, 128, SC, Q] so partition reads are contiguous
        qT_c = np.ascontiguousarray(
            qflat[:, c0:c0 + DSH].T.reshape(NJ, SC, 128, Q)
            .transpose(0, 2, 1, 3)).astype(fp8)
        # DoubleRow weight layout: (p, pair, ko, m), d = (pair*2+ko)*128 + p,
        # targets at m = 0..B-1, zero-padded to 16 columns
        tT_c = np.zeros((128, DSH // 256, 2, 16), dtype=np.float32)
        tT_c[:, :, :, :B] = (tflat[:, c0:c0 + DSH]
                             .reshape(B, DSH // 256, 2, 128)
                             .transpose(3, 1, 2, 0))
        in_maps1.append({"qT": qT_c, "tT": tT_c.astype(fp8)})
    t_prep1 = time.perf_counter()

    res1, l1_wall = _run(nc1, in_maps1)

    parts = np.stack([r["part"] for r in res1]).astype(np.float64)  # [8,3,Q]
    psum = parts.sum(axis=0)
    qn_dev = psum[0]
    dots_dev = psum[1:3]  # [B, Q] -- dots_dev[b, r] = q_r . t_b

    # ------------- host glue: exact selection (tiny, fp64) ---------------
    t64 = tflat.astype(np.float64)
    tn = (t64 ** 2).sum(axis=1)
    i_min = np.zeros(B, dtype=np.int64)
    cd = np.zeros(B)
    for b in range(B):
        d2_approx = qn_dev - 2.0 * dots_dev[b] + tn[b]
        cands = np.argsort(d2_approx, kind="stable")[:N_CAND]
        qc = qflat[cands].astype(np.float64)
        exact = ((qc - t64[b]) ** 2).sum(axis=1)
        j = int(np.argmin(exact))
        i_min[b] = cands[j]
        cd[b] = np.sqrt(exact[j])

    af64 = auged_feat.reshape(B, A, D).astype(np.float64)
    idx = np.zeros((B, k), dtype=np.int64)
    wsel = np.zeros((B, k), dtype=np.float32)
    counts = np.zeros(B, dtype=np.int64)
    denom = np.ones(B, dtype=np.float32)
    for b in range(B):
        closest = qflat[i_min[b]].astype(np.float64)
        d2 = np.sqrt(((af64[b] - closest[None, :]) ** 2).sum(axis=1))  # [A]
        mask = d2 <= cd[b]
        counts[b] = int(mask.sum())
        masked = np.where(mask, d2, np.inf)
        order = np.argsort(masked, kind="stable")[:k]
        idx[b] = order
        wsel[b] = np.isfinite(masked[order]).astype(np.float32)
        denom[b] = max(float(wsel[b].sum()), 1.0)

    # ---------------- L2: weighted softmax average (device) ----------------
    t_prep2 = time.perf_counter()
    nc2 = _get_module("l2", k)
    sel = auged_logits[np.arange(B)[:, None], idx]  # [B, k, C, H, W]
    eye = np.eye(128, dtype=np.float32)
    wI_np = (wsel.reshape(B * k, 1, 1) * eye[None]).astype(np.float32)
    in_maps2 = []
    for c in range(N_CORES):
        h0 = c * HSH
        g_c = np.ascontiguousarray(
            sel[:, :, :, h0:h0 + HSH, :].transpose(0, 1, 3, 4, 2)
        ).reshape(B * k, PXS, C)
        tg_c = np.ascontiguousarray(
            tgt_logits[:, :, h0:h0 + HSH, :].transpose(0, 2, 3, 1)
        ).reshape(B, PXS, C)
        in_maps2.append({"g": g_c, "wI": wI_np, "tg": tg_c})
    t_prep3 = time.perf_counter()

    res2, l2_wall = _run(nc2, in_maps2)

    # ---------------- host: assemble + final label logic ----------------
    U = np.empty((B, H, W, C), dtype=np.float32)
    conf = np.empty((B, H, W), dtype=np.float32)
    for c in range(N_CORES):
        h0 = c * HSH
        U[:, h0:h0 + HSH] = res2[c]["u"].reshape(B, HSH, W, C)
        conf[:, h0:h0 + HSH] = res2[c]["conf"].reshape(B, HSH, W)

    avg_soft = np.ascontiguousarray(
        (U / denom[:, None, None, None]).transpose(0, 3, 1, 2))  # [B, C, H, W]

    knn = np.argmax(avg_soft, axis=1).astype(pseudo_label.dtype)
    labels = np.where((counts > 0)[:, None, None], knn, pseudo_label)
    pseudo_mask = conf > 0.5
    refined = np.where(pseudo_mask, labels, pseudo_label).astype(pseudo_label.dtype)

    LAST_TIMING.update(dict(
        l1_wall=l1_wall, l2_wall=l2_wall,
        prep1=t_prep1 - t_prep0, prep2=t_prep3 - t_prep2))

    return refined, avg_soft


# revision 32
# speedup vs baseline: 1.1626x; 1.0689x over previous
"""Trainium2 Bass kernel for AdaptivePseudoLabelRefinement (retrieval-KNN).

Pipeline (8 NeuronCores, full inputs in / full outputs out):
  L1 (device, feature-dim sharded): stream the 512x131072 feature queue
      (fp8e4, selection-only precision) through the PE array to get, per
      core, partial [q_norm, dot(q, t0), dot(q, t1)] over its
      16384-feature shard. Dots use DoubleRow fp8 matmuls (col-group 0),
      norms a concurrent ones-matmul in col-group 1.
  host glue (tiny): sum partials -> approx distances -> top-128
      candidate rows -> exact fp64 re-verification -> argmin row and
      closest distance; exact d2 of the 32 augmented features vs the
      closest row -> mask / top-k / weights exactly mirroring the
      reference semantics.
  L2 (device, spatial-row sharded): for the k selected augmentations,
      softmax over classes (pixel-major layout) and 0/1-weighted
      PSUM accumulation -> unnormalized average U; also the target-logits
      confidence mask. Host divides by max(count,1), argmaxes, and
      assembles the refined labels exactly like the reference.
"""

import os
import time
import numpy as np

import ml_dtypes

Q = 512          # queue rows
D = 512 * 16 * 16  # 131072 feature dim
B = 2            # batch
A = 16           # augmentations
C = 19           # classes
H = 256
W = 256
N_CORES = 8
DSH = D // N_CORES      # 16384 features per core (L1 shard)
HSH = H // N_CORES      # 32 rows per core (L2 shard)
PXS = HSH * W           # 8192 pixels per core per (b, slab)
L = PXS // 128          # 64 pixel groups per partition
REFINE_CONF = 0.968
N_CAND = 128            # candidate rows re-verified exactly on host

_MODULES = {}
LAST_TIMING = {}


def _bass_imports():
    import concourse.bacc as bacc
    import concourse.mybir as mybir
    import concourse.tile as tile
    from concourse import bass_utils
    from concourse.bass_interp import get_hw_module
    return bacc, mybir, tile, bass_utils, get_hw_module


def _build_l1():
    """Queue-distance partials. Per core: qT [NJ, 128, SC, Q] fp8e4
    (transposed queue shard in on-chip tile order), tT [128, NPAIR, 2,
    PADM] fp8e4 (DoubleRow-interleaved target shard). Output part [3, Q]
    f32 = [q_norm, dot_t0, dot_t1].

    The dot (DoubleRow) and norm matmul streams run concurrently in
    distinct PE column-groups (tile_position col 0 vs 32), and the queue
    streams in fp8 (selection-only precision: the host re-verifies the
    top candidates exactly)."""
    bacc, mybir, tile, bass_utils, get_hw_module = _bass_imports()
    fp8 = mybir.dt.float8e4
    f32 = mybir.dt.float32

    NJ = 16                    # DMA loads
    SC = DSH // 128 // NJ      # 8 d-chunks of 128 per load
    NCHUNK = DSH // 128        # 128
    NPAIR = NCHUNK // 2        # DoubleRow processes chunk pairs
    # squares: ACT takes the first 5/4 chunks of each load as one coarse
    # instruction (amortizing its 224-cycle issue overhead), DVE the rest
    # in 2-chunk instructions (its overhead is small, finer grain pipelines
    # better); 9/16 on ACT balances 33.7us ACT vs 30.8us DVE busy
    ACT_PAT = (5, 4)
    DVE_GRAIN = 2

    PADM = 16                  # DoubleRow weight Ko-step must be 16 bytes

    nc = bacc.Bacc("TRN2", target_bir_lowering=False, debug=False,
                   num_devices=N_CORES)
    # host pre-arranges the queue in on-chip tile order so every partition
    # reads one fully contiguous 4KB run per load
    qT = nc.dram_tensor("qT", [NJ, 128, SC, Q], fp8, kind="ExternalInput")
    # target shard interleaved for DoubleRow: (p, pair, ko, m) with the two
    # targets in m = 0, 1 and zero padding up to PADM
    tT = nc.dram_tensor("tT", [128, NPAIR, 2, PADM], fp8,
                        kind="ExternalInput")
    out = nc.dram_tensor("part", [3, Q], f32, kind="ExternalOutput")
    PM = mybir.MatmulPerfMode.DoubleRow

    with tile.TileContext(nc) as tc:
        with (
            tc.tile_pool(name="const", bufs=1) as constp,
            tc.tile_pool(name="qload", bufs=5) as qp,
            tc.tile_pool(name="sqp", bufs=5) as sqp,
            tc.tile_pool(name="psum", bufs=1, space="PSUM") as pp,
            tc.tile_pool(name="outp", bufs=1) as op,
        ):
            tTs = constp.tile([128, NPAIR, 2, PADM], fp8)
            nc.sync.dma_start(tTs[:], tT.ap())
            ones = constp.tile([128, 1], fp8)
            nc.vector.memset(ones[:], 1.0)

            # dots: DoubleRow (2 chunks per matmul) into partitions 0-15 of
            # PE col-group 0; norms: plain fp8 ones-matmul into partition 32
            # (col-group 1) so the two streams overlap in separate column
            # strips of the array. Separate PSUM banks (one accumulation
            # group per bank).
            psd = pp.tile([PADM, Q], f32, tag="psd")
            psn = pp.tile([33, Q], f32, tag="psn")

            qv = qT.ap()
            for j in range(NJ):
                tq = qp.tile([128, SC, Q], fp8)
                nc.sync.dma_start(tq[:], qv[j])
                sq = sqp.tile([128, SC, Q], fp8)
                ca = ACT_PAT[j % len(ACT_PAT)]
                nc.scalar.square(sq[:, 0:ca], tq[:, 0:ca])
                z = ca
                while z < SC:
                    z1 = min(z + DVE_GRAIN, SC)
                    nc.vector.tensor_mul(sq[:, z:z1], tq[:, z:z1],
                                         tq[:, z:z1])
                    z = z1
                for s in range(0, SC, 2):
                    pi = (j * SC + s) // 2
                    nc.tensor.matmul(psd[:, :], tTs[:, pi, :, :],
                                     tq[:, s:s + 2, :],
                                     start=(pi == 0), stop=(pi == NPAIR - 1),
                                     perf_mode=PM, tile_position=(0, 0))
                for s in range(SC):
                    ci = j * SC + s
                    nc.tensor.matmul(psn[32:33, :], ones[:], sq[:, s, :],
                                     start=(ci == 0), stop=(ci == NCHUNK - 1),
                                     tile_position=(0, 32))

            outs_d = op.tile([2, Q], f32, tag="outd")
            nc.vector.tensor_copy(outs_d[:], psd[0:B, :])
            nc.sync.dma_start(out.ap()[1:3], outs_d[:])
            outs_n = op.tile([33, Q], f32, tag="outn")
            nc.vector.tensor_copy(outs_n[32:33, :], psn[32:33, :])
            nc.sync.dma_start(out.ap()[0:1], outs_n[32:33, :])

    nc.compile()
    return nc


def _build_l2(k):
    """Weighted softmax average over the k selected augmentations plus the
    target-logits confidence mask, pixel-major layout, H-sharded.

    Per core: g [B*k, PXS, C] f32 (gathered aug logits), wI [B*k, 128, 128]
    f32 (weight * identity, stationary for exact fp32 PSUM accumulation),
    tg [B, PXS, C] f32 (target logits).
    Outputs: u [B, PXS, C] f32 (sum_j w_j * softmax_j), conf [B, PXS] f32
    (1.0 where max softmax < REFINE_CONF)."""
    bacc, mybir, tile, bass_utils, get_hw_module = _bass_imports()
    f32 = mybir.dt.float32
    AF = mybir.ActivationFunctionType
    ALU = mybir.AluOpType
    AX = mybir.AxisListType
    S = B * k

    nc = bacc.Bacc("TRN2", target_bir_lowering=False, debug=False,
                   num_devices=N_CORES)
    g = nc.dram_tensor("g", [S, PXS, C], f32, kind="ExternalInput")
    wI = nc.dram_tensor("wI", [S, 128, 128], f32, kind="ExternalInput")
    tg = nc.dram_tensor("tg", [B, PXS, C], f32, kind="ExternalInput")
    u = nc.dram_tensor("u", [B, PXS, C], f32, kind="ExternalOutput")
    cf = nc.dram_tensor("conf", [B, PXS], f32, kind="ExternalOutput")

    gview = g.ap().rearrange("s (p l) c -> s p l c", p=128)
    tview = tg.ap().rearrange("b (p l) c -> b p l c", p=128)
    uview = u.ap().rearrange("b (p l) c -> b p l c", p=128)
    cview = cf.ap().rearrange("b (p l) -> b p l", p=128)
    FD = L * C  # 1216 accumulator columns per partition
    # sub-slab pipelining: normalize and accumulate in l-aligned thirds so
    # the PE accumulation of one third overlaps the DVE normalize of the
    # next. Each third's PSUM region starts on its own bank (512-f32
    # aligned) because a matmul output cannot cross a bank boundary.
    LSPLIT = [(0, 26), (26, 52), (52, L)]
    BANK = 512
    ACC_OFF = [0 * BANK, 1 * BANK, 2 * BANK]

    with tile.TileContext(nc) as tc:
        with (
            tc.tile_pool(name="const", bufs=1) as constp,
            tc.tile_pool(name="raw", bufs=3) as rawp,
            tc.tile_pool(name="exp", bufs=3) as ep,
            tc.tile_pool(name="small", bufs=4) as sp,
            tc.tile_pool(name="smp", bufs=4) as smp,
            tc.tile_pool(name="psum", bufs=1, space="PSUM") as pp,
            tc.tile_pool(name="accp", bufs=2) as accp,
        ):
            # per-slab 0/1-weighted identity: stationary for PE accumulation
            wIs = constp.tile([128, S, 128], f32)
            nc.sync.dma_start(wIs[:], wI.ap().rearrange("s p m -> p s m"))

            accs = {}
            for b in range(B):
                acc_t = pp.tile([128, 2 * BANK + (FD - 52 * C)], f32,
                                tag=f"acc_{b}")
                accs[b] = acc_t

            # interleave the two batch samples' slab streams so their PSUM
            # accumulations and DVE chains overlap
            for j in range(k):
                for b in range(B):
                    s = b * k + j
                    raw = rawp.tile([128, L, C], f32)
                    nc.sync.dma_start(raw[:], gview[s])
                    E = ep.tile([128, L, C], f32)
                    nc.scalar.activation(E[:], raw[:], AF.Exp)
                    Ssum = sp.tile([128, L], f32)
                    R = sp.tile([128, L], f32, tag="recip")
                    sm = smp.tile([128, L, C], f32)
                    sm_flat = sm[:].rearrange("p l c -> p (l c)")
                    for t, (l0, l1) in enumerate(LSPLIT):
                        nc.vector.tensor_reduce(Ssum[:, l0:l1],
                                                E[:, l0:l1, :], AX.X, ALU.add)
                        nc.vector.reciprocal(R[:, l0:l1], Ssum[:, l0:l1])
                        rb = R[:, l0:l1].unsqueeze(-1).broadcast_to(
                            [128, l1 - l0, C])
                        nc.vector.tensor_tensor(sm[:, l0:l1, :],
                                                E[:, l0:l1, :], rb, ALU.mult)
                        o = ACC_OFF[t]
                        nc.tensor.matmul(
                            accs[b][:, o:o + (l1 - l0) * C],
                            wIs[:, s, :], sm_flat[:, l0 * C:l1 * C],
                            start=(j == 0), stop=(j == k - 1))

            for b in range(B):
                Uacc = accp.tile([128, FD], f32)
                for t, (l0, l1) in enumerate(LSPLIT):
                    o = ACC_OFF[t]
                    # ACT does the PSUM->SBUF copies; DVE is the busy engine
                    nc.scalar.copy(
                        Uacc[:, l0 * C:l1 * C],
                        accs[b][:, o:o + (l1 - l0) * C])
                nc.sync.dma_start(
                    uview[b], Uacc[:].rearrange("p (l c) -> p l c", c=C))

            # target-logits confidence: 1.0 where max softmax < conf
            for b in range(B):
                rawt = rawp.tile([128, L, C], f32, tag="rawt")
                nc.sync.dma_start(rawt[:], tview[b])
                Et = ep.tile([128, L, C], f32, tag="Et")
                nc.scalar.activation(Et[:], rawt[:], AF.Exp)
                mx = sp.tile([128, L], f32, tag="mx")
                nc.vector.tensor_reduce(mx[:], Et[:], AX.X, ALU.max)
                sm_ = sp.tile([128, L], f32, tag="sum")
                nc.vector.tensor_reduce(sm_[:], Et[:], AX.X, ALU.add)
                cmpo = sp.tile([128, L], f32, tag="cmp")
                nc.vector.scalar_tensor_tensor(
                    cmpo[:], sm_[:], REFINE_CONF, mx[:], ALU.mult, ALU.is_gt)
                nc.sync.dma_start(cview[b], cmpo[:])

    nc.compile()
    return nc


def _get_module(name, k=None):
    key = (name, k)
    if key not in _MODULES:
        if name == "l1":
            _MODULES[key] = _build_l1()
        else:
            _MODULES[key] = _build_l2(k)
    return _MODULES[key]


def _run_hw(nc, in_maps):
    """Run the compiled module on the 8 NeuronCores via PJRT/axon."""
    _, _, _, bass_utils, get_hw_module = _bass_imports()
    from concourse.bass_interp import get_hw_module as _ghm
    old_m = nc.m
    nc.m = _ghm(nc.m)
    try:
        t0 = time.perf_counter()
        res = bass_utils.run_bass_kernel_spmd(
            nc, in_maps, core_ids=list(range(N_CORES)))
        t1 = time.perf_counter()
    finally:
        nc.m = old_m
    return res.results, t1 - t0


def _run_sim(nc, in_maps):
    """CoreSim fallback (KERNEL_BACKEND=sim) for development."""
    from concourse.bass_interp import CoreSim
    results = []
    for m in in_maps:
        sim = CoreSim(nc, require_finite=False)
        for name, arr in m.items():
            sim.tensor(name)[:] = arr
        sim.simulate(check_with_hw=False)
        outs = {}
        for alloc in nc.m.functions[0].allocations:
            try:
                kind = alloc.kind
            except AttributeError:
                continue
            if kind == "ExternalOutput":
                nm = alloc.memorylocations[0].name
                outs[nm] = np.array(sim.tensor(nm))
        results.append(outs)
    return results, 0.0


def _run(nc, in_maps):
    if os.environ.get("KERNEL_BACKEND") == "sim":
        return _run_sim(nc, in_maps)
    return _run_hw(nc, in_maps)


def kernel(source_queue, tgt_feat, tgt_logits, auged_feat, auged_logits,
           pseudo_label, k):
    k = int(k)
    fp8 = ml_dtypes.float8_e4m3

    source_queue = np.ascontiguousarray(np.asarray(source_queue, dtype=np.float32))
    tgt_feat = np.asarray(tgt_feat, dtype=np.float32)
    tgt_logits = np.asarray(tgt_logits, dtype=np.float32)
    auged_feat = np.asarray(auged_feat, dtype=np.float32)
    auged_logits = np.ascontiguousarray(np.asarray(auged_logits, dtype=np.float32))
    pseudo_label = np.asarray(pseudo_label)

    qflat = source_queue.reshape(Q, D)
    tflat = tgt_feat.reshape(B, D)

    # ---------------- L1: queue distance partials (device) ----------------
    t_prep0 = time.perf_counter()
    nc1 = _get_module("l1")
    in_maps1 = []
    NJ = 16
    SC = DSH // 128 // NJ
    for c in range(N_CORES):
        c0 = c * DSH
        # [DSH, Q] transposed shard, rows d = j*(SC*128) + s*128 + p,
        # rearranged to [NJ, 128, SC, Q] so partition reads are contiguous
        qT_c = np.ascontiguousarray(
            qflat[:, c0:c0 + DSH].T.reshape(NJ, SC, 128, Q)
            .transpose(0, 2, 1, 3)).astype(fp8)
        # DoubleRow weight layout: (p, pair, ko, m), d = (pair*2+ko)*128 + p,
        # targets at m = 0..B-1, zero-padded to 16 columns
        tT_c = np.zeros((128, DSH // 256, 2, 16), dtype=np.float32)
        tT_c[:, :, :, :B] = (tflat[:, c0:c0 + DSH]
                             .reshape(B, DSH // 256, 2, 128)
                             .transpose(3, 1, 2, 0))
        in_maps1.append({"qT": qT_c, "tT": tT_c.astype(fp8)})
    t_prep1 = time.perf_counter()

    res1, l1_wall = _run(nc1, in_maps1)

    parts = np.stack([r["part"] for r in res1]).astype(np.float64)  # [8,3,Q]
    psum = parts.sum(axis=0)
    qn_dev = psum[0]
    dots_dev = psum[1:3]  # [B, Q] -- dots_dev[b, r] = q_r . t_b

    # ------------- host glue: exact selection (tiny, fp64) ---------------
    t64 = tflat.astype(np.float64)
    tn = (t64 ** 2).sum(axis=1)
    i_min = np.zeros(B, dtype=np.int64)
    cd = np.zeros(B)
    for b in range(B):
        d2_approx = qn_dev - 2.0 * dots_dev[b] + tn[b]
        cands = np.argsort(d2_approx, kind="stable")[:N_CAND]
        qc = qflat[cands].astype(np.float64)
        exact = ((qc - t64[b]) ** 2).sum(axis=1)
        j = int(np.argmin(exact))
        i_min[b] = cands[j]
        cd[b] = np.sqrt(exact[j])

    af64 = auged_feat.reshape(B, A, D).astype(np.float64)
    idx = np.zeros((B, k), dtype=np.int64)
    wsel = np.zeros((B, k), dtype=np.float32)
    counts = np.zeros(B, dtype=np.int64)
    denom = np.ones(B, dtype=np.float32)
    for b in range(B):
        closest = qflat[i_min[b]].astype(np.float64)
        d2 = np.sqrt(((af64[b] - closest[None, :]) ** 2).sum(axis=1))  # [A]
        mask = d2 <= cd[b]
        counts[b] = int(mask.sum())
        masked = np.where(mask, d2, np.inf)
        order = np.argsort(masked, kind="stable")[:k]
        idx[b] = order
        wsel[b] = np.isfinite(masked[order]).astype(np.float32)
        denom[b] = max(float(wsel[b].sum()), 1.0)

    # ---------------- L2: weighted softmax average (device) ----------------
    t_prep2 = time.perf_counter()
    nc2 = _get_module("l2", k)
    sel = auged_logits[np.arange(B)[:, None], idx]  # [B, k, C, H, W]
    eye = np.eye(128, dtype=np.float32)
    wI_np = (wsel.reshape(B * k, 1, 1) * eye[None]).astype(np.float32)
    in_maps2 = []
    for c in range(N_CORES):
        h0 = c * HSH
        g_c = np.ascontiguousarray(
            sel[:, :, :, h0:h0 + HSH, :].transpose(0, 1, 3, 4, 2)
        ).reshape(B * k, PXS, C)
        tg_c = np.ascontiguousarray(
            tgt_logits[:, :, h0:h0 + HSH, :].transpose(0, 2, 3, 1)
        ).reshape(B, PXS, C)
        in_maps2.append({"g": g_c, "wI": wI_np, "tg": tg_c})
    t_prep3 = time.perf_counter()

    res2, l2_wall = _run(nc2, in_maps2)

    # ---------------- host: assemble + final label logic ----------------
    U = np.empty((B, H, W, C), dtype=np.float32)
    conf = np.empty((B, H, W), dtype=np.float32)
    for c in range(N_CORES):
        h0 = c * HSH
        U[:, h0:h0 + HSH] = res2[c]["u"].reshape(B, HSH, W, C)
        conf[:, h0:h0 + HSH] = res2[c]["conf"].reshape(B, HSH, W)

    avg_soft = np.ascontiguousarray(
        (U / denom[:, None, None, None]).transpose(0, 3, 1, 2))  # [B, C, H, W]

    knn = np.argmax(avg_soft, axis=1).astype(pseudo_label.dtype)
    labels = np.where((counts > 0)[:, None, None], knn, pseudo_label)
    pseudo_mask = conf > 0.5
    refined = np.where(pseudo_mask, labels, pseudo_label).astype(pseudo_label.dtype)

    LAST_TIMING.update(dict(
        l1_wall=l1_wall, l2_wall=l2_wall,
        prep1=t_prep1 - t_prep0, prep2=t_prep3 - t_prep2))

    return refined, avg_soft


# revision 33
# speedup vs baseline: 1.1803x; 1.0152x over previous
"""Trainium2 Bass kernel for AdaptivePseudoLabelRefinement (retrieval-KNN).

Pipeline (8 NeuronCores, full inputs in / full outputs out):
  L1 (device, feature-dim sharded): stream the 512x131072 feature queue
      (fp8e4, selection-only precision) through the PE array to get, per
      core, partial [q_norm, dot(q, t0), dot(q, t1)] over its
      16384-feature shard. Dots use DoubleRow fp8 matmuls (col-group 0),
      norms a concurrent ones-matmul in col-group 1.
  host glue (tiny): sum partials -> approx distances -> top-128
      candidate rows -> exact fp64 re-verification -> argmin row and
      closest distance; exact d2 of the 32 augmented features vs the
      closest row -> mask / top-k / weights exactly mirroring the
      reference semantics.
  L2 (device, spatial-row sharded): for the k selected augmentations,
      softmax over classes (pixel-major layout) and 0/1-weighted
      PSUM accumulation -> unnormalized average U; also the target-logits
      confidence mask. Host divides by max(count,1), argmaxes, and
      assembles the refined labels exactly like the reference.
"""

import os
import time
import numpy as np

import ml_dtypes

Q = 512          # queue rows
D = 512 * 16 * 16  # 131072 feature dim
B = 2            # batch
A = 16           # augmentations
C = 19           # classes
H = 256
W = 256
N_CORES = 8
DSH = D // N_CORES      # 16384 features per core (L1 shard)
HSH = H // N_CORES      # 32 rows per core (L2 shard)
PXS = HSH * W           # 8192 pixels per core per (b, slab)
L = PXS // 128          # 64 pixel groups per partition
REFINE_CONF = 0.968
N_CAND = 128            # candidate rows re-verified exactly on host

_MODULES = {}
LAST_TIMING = {}


def _bass_imports():
    import concourse.bacc as bacc
    import concourse.mybir as mybir
    import concourse.tile as tile
    from concourse import bass_utils
    from concourse.bass_interp import get_hw_module
    return bacc, mybir, tile, bass_utils, get_hw_module


def _build_l1():
    """Queue-distance partials. Per core: qT [NJ, 128, SC, Q] fp8e4
    (transposed queue shard in on-chip tile order), tT [128, NPAIR, 2,
    PADM] fp8e4 (DoubleRow-interleaved target shard). Output part [3, Q]
    f32 = [q_norm, dot_t0, dot_t1].

    The dot (DoubleRow) and norm matmul streams run concurrently in
    distinct PE column-groups (tile_position col 0 vs 32), and the queue
    streams in fp8 (selection-only precision: the host re-verifies the
    top candidates exactly)."""
    bacc, mybir, tile, bass_utils, get_hw_module = _bass_imports()
    fp8 = mybir.dt.float8e4
    f32 = mybir.dt.float32

    NJ = 16                    # DMA loads
    SC = DSH // 128 // NJ      # 8 d-chunks of 128 per load
    NCHUNK = DSH // 128        # 128
    NPAIR = NCHUNK // 2        # DoubleRow processes chunk pairs
    # squares: ACT takes the first 5/4 chunks of each load as one coarse
    # instruction (amortizing its 224-cycle issue overhead), DVE the rest
    # in 2-chunk instructions (its overhead is small, finer grain pipelines
    # better); 9/16 on ACT balances 33.7us ACT vs 30.8us DVE busy
    ACT_PAT = (5, 4)
    DVE_GRAIN = 2

    PADM = 16                  # DoubleRow weight Ko-step must be 16 bytes

    nc = bacc.Bacc("TRN2", target_bir_lowering=False, debug=False,
                   num_devices=N_CORES)
    # host pre-arranges the queue in on-chip tile order so every partition
    # reads one fully contiguous 4KB run per load
    qT = nc.dram_tensor("qT", [NJ, 128, SC, Q], fp8, kind="ExternalInput")
    # target shard interleaved for DoubleRow: (p, pair, ko, m) with the two
    # targets in m = 0, 1 and zero padding up to PADM
    tT = nc.dram_tensor("tT", [128, NPAIR, 2, PADM], fp8,
                        kind="ExternalInput")
    out = nc.dram_tensor("part", [3, Q], f32, kind="ExternalOutput")
    PM = mybir.MatmulPerfMode.DoubleRow

    with tile.TileContext(nc) as tc:
        with (
            tc.tile_pool(name="const", bufs=1) as constp,
            tc.tile_pool(name="qload", bufs=5) as qp,
            tc.tile_pool(name="sqp", bufs=5) as sqp,
            tc.tile_pool(name="psum", bufs=1, space="PSUM") as pp,
            tc.tile_pool(name="outp", bufs=1) as op,
        ):
            tTs = constp.tile([128, NPAIR, 2, PADM], fp8)
            nc.sync.dma_start(tTs[:], tT.ap())
            ones = constp.tile([128, 1], fp8)
            nc.vector.memset(ones[:], 1.0)

            # dots: DoubleRow (2 chunks per matmul) into partitions 0-15 of
            # PE col-group 0; norms: plain fp8 ones-matmul into partition 32
            # (col-group 1) so the two streams overlap in separate column
            # strips of the array. Separate PSUM banks (one accumulation
            # group per bank).
            psd = pp.tile([PADM, Q], f32, tag="psd")
            psn = pp.tile([33, Q], f32, tag="psn")

            qv = qT.ap()
            for j in range(NJ):
                tq = qp.tile([128, SC, Q], fp8)
                nc.sync.dma_start(tq[:], qv[j])
                sq = sqp.tile([128, SC, Q], fp8)
                ca = ACT_PAT[j % len(ACT_PAT)]
                nc.scalar.square(sq[:, 0:ca], tq[:, 0:ca])
                z = ca
                while z < SC:
                    z1 = min(z + DVE_GRAIN, SC)
                    nc.vector.tensor_mul(sq[:, z:z1], tq[:, z:z1],
                                         tq[:, z:z1])
                    z = z1
                for s in range(0, SC, 2):
                    pi = (j * SC + s) // 2
                    nc.tensor.matmul(psd[:, :], tTs[:, pi, :, :],
                                     tq[:, s:s + 2, :],
                                     start=(pi == 0), stop=(pi == NPAIR - 1),
                                     perf_mode=PM, tile_position=(0, 0))
                for s in range(SC):
                    ci = j * SC + s
                    nc.tensor.matmul(psn[32:33, :], ones[:], sq[:, s, :],
                                     start=(ci == 0), stop=(ci == NCHUNK - 1),
                                     tile_position=(0, 32))

            outs_d = op.tile([2, Q], f32, tag="outd")
            nc.vector.tensor_copy(outs_d[:], psd[0:B, :])
            nc.sync.dma_start(out.ap()[1:3], outs_d[:])
            outs_n = op.tile([33, Q], f32, tag="outn")
            nc.vector.tensor_copy(outs_n[32:33, :], psn[32:33, :])
            nc.sync.dma_start(out.ap()[0:1], outs_n[32:33, :])

    nc.compile()
    return nc


def _build_l2(k):
    """Weighted softmax average over the k selected augmentations plus the
    target-logits confidence mask, pixel-major layout, H-sharded.

    Per core: g [B*k, PXS, C] f32 (gathered aug logits), wI [B*k, 128, 128]
    f32 (weight * identity, stationary for exact fp32 PSUM accumulation),
    tg [B, PXS, C] f32 (target logits).
    Outputs: u [B, PXS, C] f32 (sum_j w_j * softmax_j), conf [B, PXS] f32
    (1.0 where max softmax < REFINE_CONF)."""
    bacc, mybir, tile, bass_utils, get_hw_module = _bass_imports()
    f32 = mybir.dt.float32
    AF = mybir.ActivationFunctionType
    ALU = mybir.AluOpType
    AX = mybir.AxisListType
    S = B * k

    nc = bacc.Bacc("TRN2", target_bir_lowering=False, debug=False,
                   num_devices=N_CORES)
    g = nc.dram_tensor("g", [S, PXS, C], f32, kind="ExternalInput")
    wI = nc.dram_tensor("wI", [S, 128, 128], f32, kind="ExternalInput")
    tg = nc.dram_tensor("tg", [B, PXS, C], f32, kind="ExternalInput")
    u = nc.dram_tensor("u", [B, PXS, C], f32, kind="ExternalOutput")
    cf = nc.dram_tensor("conf", [B, PXS], f32, kind="ExternalOutput")

    gview = g.ap().rearrange("s (p l) c -> s p l c", p=128)
    tview = tg.ap().rearrange("b (p l) c -> b p l c", p=128)
    uview = u.ap().rearrange("b (p l) c -> b p l c", p=128)
    cview = cf.ap().rearrange("b (p l) -> b p l", p=128)
    FD = L * C  # 1216 accumulator columns per partition
    # sub-slab pipelining: normalize and accumulate in l-aligned thirds so
    # the PE accumulation of one third overlaps the DVE normalize of the
    # next. Each third's PSUM region starts on its own bank (512-f32
    # aligned) because a matmul output cannot cross a bank boundary.
    LSPLIT = [(0, 26), (26, 52), (52, L)]
    BANK = 512
    ACC_OFF = [0 * BANK, 1 * BANK, 2 * BANK]

    with tile.TileContext(nc) as tc:
        with (
            tc.tile_pool(name="const", bufs=1) as constp,
            tc.tile_pool(name="raw", bufs=3) as rawp,
            tc.tile_pool(name="exp", bufs=3) as ep,
            tc.tile_pool(name="small", bufs=4) as sp,
            tc.tile_pool(name="smp", bufs=4) as smp,
            tc.tile_pool(name="psum", bufs=1, space="PSUM") as pp,
            tc.tile_pool(name="accp", bufs=2) as accp,
        ):
            # per-slab 0/1-weighted identity: stationary for PE accumulation
            wIs = constp.tile([128, S, 128], f32)
            nc.sync.dma_start(wIs[:], wI.ap().rearrange("s p m -> p s m"))

            accs = {}
            for b in range(B):
                acc_t = pp.tile([128, 2 * BANK + (FD - 52 * C)], f32,
                                tag=f"acc_{b}")
                accs[b] = acc_t

            # interleave the two batch samples' slab streams so their PSUM
            # accumulations and DVE chains overlap
            for j in range(k):
                for b in range(B):
                    s = b * k + j
                    raw = rawp.tile([128, L, C], f32)
                    nc.sync.dma_start(raw[:], gview[s])
                    E = ep.tile([128, L, C], f32)
                    nc.scalar.activation(E[:], raw[:], AF.Exp)
                    Ssum = sp.tile([128, L], f32)
                    R = sp.tile([128, L], f32, tag="recip")
                    sm = smp.tile([128, L, C], f32)
                    sm_flat = sm[:].rearrange("p l c -> p (l c)")
                    for t, (l0, l1) in enumerate(LSPLIT):
                        nc.vector.tensor_reduce(Ssum[:, l0:l1],
                                                E[:, l0:l1, :], AX.X, ALU.add)
                        nc.vector.reciprocal(R[:, l0:l1], Ssum[:, l0:l1])
                        rb = R[:, l0:l1].unsqueeze(-1).broadcast_to(
                            [128, l1 - l0, C])
                        # middle third's normalize runs on GpSimd: DVE's
                        # reduce/reciprocal are single-tensor ops on its
                        # dedicated SBUF ports, so Pool gets the shared pair
                        eng = nc.gpsimd if t == 1 else nc.vector
                        eng.tensor_tensor(sm[:, l0:l1, :],
                                          E[:, l0:l1, :], rb, ALU.mult)
                        o = ACC_OFF[t]
                        nc.tensor.matmul(
                            accs[b][:, o:o + (l1 - l0) * C],
                            wIs[:, s, :], sm_flat[:, l0 * C:l1 * C],
                            start=(j == 0), stop=(j == k - 1))

            for b in range(B):
                Uacc = accp.tile([128, FD], f32)
                for t, (l0, l1) in enumerate(LSPLIT):
                    o = ACC_OFF[t]
                    # ACT does the PSUM->SBUF copies; DVE is the busy engine
                    nc.scalar.copy(
                        Uacc[:, l0 * C:l1 * C],
                        accs[b][:, o:o + (l1 - l0) * C])
                nc.sync.dma_start(
                    uview[b], Uacc[:].rearrange("p (l c) -> p l c", c=C))

            # target-logits confidence: 1.0 where max softmax < conf
            for b in range(B):
                rawt = rawp.tile([128, L, C], f32, tag="rawt")
                nc.sync.dma_start(rawt[:], tview[b])
                Et = ep.tile([128, L, C], f32, tag="Et")
                nc.scalar.activation(Et[:], rawt[:], AF.Exp)
                mx = sp.tile([128, L], f32, tag="mx")
                nc.vector.tensor_reduce(mx[:], Et[:], AX.X, ALU.max)
                sm_ = sp.tile([128, L], f32, tag="sum")
                nc.vector.tensor_reduce(sm_[:], Et[:], AX.X, ALU.add)
                cmpo = sp.tile([128, L], f32, tag="cmp")
                nc.vector.scalar_tensor_tensor(
                    cmpo[:], sm_[:], REFINE_CONF, mx[:], ALU.mult, ALU.is_gt)
                nc.sync.dma_start(cview[b], cmpo[:])

    nc.compile()
    return nc


def _get_module(name, k=None):
    key = (name, k)
    if key not in _MODULES:
        if name == "l1":
            _MODULES[key] = _build_l1()
        else:
            _MODULES[key] = _build_l2(k)
    return _MODULES[key]


def _run_hw(nc, in_maps):
    """Run the compiled module on the 8 NeuronCores via PJRT/axon."""
    _, _, _, bass_utils, get_hw_module = _bass_imports()
    from concourse.bass_interp import get_hw_module as _ghm
    old_m = nc.m
    nc.m = _ghm(nc.m)
    try:
        t0 = time.perf_counter()
        res = bass_utils.run_bass_kernel_spmd(
            nc, in_maps, core_ids=list(range(N_CORES)))
        t1 = time.perf_counter()
    finally:
        nc.m = old_m
    return res.results, t1 - t0


def _run_sim(nc, in_maps):
    """CoreSim fallback (KERNEL_BACKEND=sim) for development."""
    from concourse.bass_interp import CoreSim
    results = []
    for m in in_maps:
        sim = CoreSim(nc, require_finite=False)
        for name, arr in m.items():
            sim.tensor(name)[:] = arr
        sim.simulate(check_with_hw=False)
        outs = {}
        for alloc in nc.m.functions[0].allocations:
            try:
                kind = alloc.kind
            except AttributeError:
                continue
            if kind == "ExternalOutput":
                nm = alloc.memorylocations[0].name
                outs[nm] = np.array(sim.tensor(nm))
        results.append(outs)
    return results, 0.0


def _run(nc, in_maps):
    if os.environ.get("KERNEL_BACKEND") == "sim":
        return _run_sim(nc, in_maps)
    return _run_hw(nc, in_maps)


def kernel(source_queue, tgt_feat, tgt_logits, auged_feat, auged_logits,
           pseudo_label, k):
    k = int(k)
    fp8 = ml_dtypes.float8_e4m3

    source_queue = np.ascontiguousarray(np.asarray(source_queue, dtype=np.float32))
    tgt_feat = np.asarray(tgt_feat, dtype=np.float32)
    tgt_logits = np.asarray(tgt_logits, dtype=np.float32)
    auged_feat = np.asarray(auged_feat, dtype=np.float32)
    auged_logits = np.ascontiguousarray(np.asarray(auged_logits, dtype=np.float32))
    pseudo_label = np.asarray(pseudo_label)

    qflat = source_queue.reshape(Q, D)
    tflat = tgt_feat.reshape(B, D)

    # ---------------- L1: queue distance partials (device) ----------------
    t_prep0 = time.perf_counter()
    nc1 = _get_module("l1")
    in_maps1 = []
    NJ = 16
    SC = DSH // 128 // NJ
    for c in range(N_CORES):
        c0 = c * DSH
        # [DSH, Q] transposed shard, rows d = j*(SC*128) + s*128 + p,
        # rearranged to [NJ, 128, SC, Q] so partition reads are contiguous
        qT_c = np.ascontiguousarray(
            qflat[:, c0:c0 + DSH].T.reshape(NJ, SC, 128, Q)
            .transpose(0, 2, 1, 3)).astype(fp8)
        # DoubleRow weight layout: (p, pair, ko, m), d = (pair*2+ko)*128 + p,
        # targets at m = 0..B-1, zero-padded to 16 columns
        tT_c = np.zeros((128, DSH // 256, 2, 16), dtype=np.float32)
        tT_c[:, :, :, :B] = (tflat[:, c0:c0 + DSH]
                             .reshape(B, DSH // 256, 2, 128)
                             .transpose(3, 1, 2, 0))
        in_maps1.append({"qT": qT_c, "tT": tT_c.astype(fp8)})
    t_prep1 = time.perf_counter()

    res1, l1_wall = _run(nc1, in_maps1)

    parts = np.stack([r["part"] for r in res1]).astype(np.float64)  # [8,3,Q]
    psum = parts.sum(axis=0)
    qn_dev = psum[0]
    dots_dev = psum[1:3]  # [B, Q] -- dots_dev[b, r] = q_r . t_b

    # ------------- host glue: exact selection (tiny, fp64) ---------------
    t64 = tflat.astype(np.float64)
    tn = (t64 ** 2).sum(axis=1)
    i_min = np.zeros(B, dtype=np.int64)
    cd = np.zeros(B)
    for b in range(B):
        d2_approx = qn_dev - 2.0 * dots_dev[b] + tn[b]
        cands = np.argsort(d2_approx, kind="stable")[:N_CAND]
        qc = qflat[cands].astype(np.float64)
        exact = ((qc - t64[b]) ** 2).sum(axis=1)
        j = int(np.argmin(exact))
        i_min[b] = cands[j]
        cd[b] = np.sqrt(exact[j])

    af64 = auged_feat.reshape(B, A, D).astype(np.float64)
    idx = np.zeros((B, k), dtype=np.int64)
    wsel = np.zeros((B, k), dtype=np.float32)
    counts = np.zeros(B, dtype=np.int64)
    denom = np.ones(B, dtype=np.float32)
    for b in range(B):
        closest = qflat[i_min[b]].astype(np.float64)
        d2 = np.sqrt(((af64[b] - closest[None, :]) ** 2).sum(axis=1))  # [A]
        mask = d2 <= cd[b]
        counts[b] = int(mask.sum())
        masked = np.where(mask, d2, np.inf)
        order = np.argsort(masked, kind="stable")[:k]
        idx[b] = order
        wsel[b] = np.isfinite(masked[order]).astype(np.float32)
        denom[b] = max(float(wsel[b].sum()), 1.0)

    # ---------------- L2: weighted softmax average (device) ----------------
    t_prep2 = time.perf_counter()
    nc2 = _get_module("l2", k)
    sel = auged_logits[np.arange(B)[:, None], idx]  # [B, k, C, H, W]
    eye = np.eye(128, dtype=np.float32)
    wI_np = (wsel.reshape(B * k, 1, 1) * eye[None]).astype(np.float32)
    in_maps2 = []
    for c in range(N_CORES):
        h0 = c * HSH
        g_c = np.ascontiguousarray(
            sel[:, :, :, h0:h0 + HSH, :].transpose(0, 1, 3, 4, 2)
        ).reshape(B * k, PXS, C)
        tg_c = np.ascontiguousarray(
            tgt_logits[:, :, h0:h0 + HSH, :].transpose(0, 2, 3, 1)
        ).reshape(B, PXS, C)
        in_maps2.append({"g": g_c, "wI": wI_np, "tg": tg_c})
    t_prep3 = time.perf_counter()

    res2, l2_wall = _run(nc2, in_maps2)

    # ---------------- host: assemble + final label logic ----------------
    U = np.empty((B, H, W, C), dtype=np.float32)
    conf = np.empty((B, H, W), dtype=np.float32)
    for c in range(N_CORES):
        h0 = c * HSH
        U[:, h0:h0 + HSH] = res2[c]["u"].reshape(B, HSH, W, C)
        conf[:, h0:h0 + HSH] = res2[c]["conf"].reshape(B, HSH, W)

    avg_soft = np.ascontiguousarray(
        (U / denom[:, None, None, None]).transpose(0, 3, 1, 2))  # [B, C, H, W]

    knn = np.argmax(avg_soft, axis=1).astype(pseudo_label.dtype)
    labels = np.where((counts > 0)[:, None, None], knn, pseudo_label)
    pseudo_mask = conf > 0.5
    refined = np.where(pseudo_mask, labels, pseudo_label).astype(pseudo_label.dtype)

    LAST_TIMING.update(dict(
        l1_wall=l1_wall, l2_wall=l2_wall,
        prep1=t_prep1 - t_prep0, prep2=t_prep3 - t_prep2))

    return refined, avg_soft
